# revision 57
# baseline (speedup 1.0000x reference)
"""Causal varlen self-attention (packed equal-length sequences) on 8 trn2 cores.

Sharding: 4 sequences x 2 head-groups. Core c handles sequence b = c//2 and
heads hh*8..hh*8+8 (hh = c%2). Each core computes the QKV projection of its
sequence restricted to its 8 heads, rotary+RMSNorm, causal attention for all
1024 rows over its heads, exchanges bf16 attention outputs with its pair
partner via AllGather, and computes the final output projection for its
512-wide column slice of y (even core: out cols 0..512, odd: 512..1024) over
the full 1024-feature contraction. The host assembles y column-wise -- the
program is SPMD-symmetric with no all-reduce.

All matmul inputs are bf16 (f32 PSUM accumulation); the host pre-transposes x
and pre-converts weights. Scores are computed in a transposed layout
[kpos, q] with causal column-trimming (matmuls/exp/PV only touch q >= kc*128);
within-diagonal-block masking zeroes est's upper triangle on the gpsimd
engine after exp. The k-side RMS norm is folded into exp's per-partition
scale; softmax denominators come from 64 ones-columns appended to V and one
vector divide per head normalizes the attention output.
"""
import numpy as np

N_EMBD = 1024
N_HEAD = 16
HD = 64
S = 1024
B = 4
N = B * S
NCORES = 8
HPC = 8            # heads per core
NHC = HPC // 2     # head-pair chunks per core
NB = S // 128      # row blocks per sequence
ND = N_EMBD // 128  # contraction chunks
JW = 3 * HPC * HD  # qkv feature width per core (1536)
OW = N_EMBD // 2   # output columns per core (512)
RMS_EPS = 1.1920929e-07

_cached = {}


def _build():
    import concourse.bacc as bacc
    import concourse.mybir as mybir
    import concourse.tile as tile
    import concourse.bass as bass
    from concourse.masks import make_identity

    F32 = mybir.dt.float32
    BF16 = mybir.dt.bfloat16
    ALU = mybir.AluOpType
    ACT = mybir.ActivationFunctionType

    nc = bacc.Bacc('TRN2', target_bir_lowering=False, debug=False,
                   num_devices=NCORES)
    xT = nc.dram_tensor('xT', [N_EMBD, S], BF16, kind='ExternalInput').ap()
    wqkvT = nc.dram_tensor('wqkvT', [N_EMBD, JW], BF16, kind='ExternalInput').ap()
    woT = nc.dram_tensor('woT', [NHC * 128, N_EMBD], BF16, kind='ExternalInput').ap()
    cosg = nc.dram_tensor('cosg', [S, HD // 2], BF16, kind='ExternalInput').ap()
    sing = nc.dram_tensor('sing', [S, HD // 2], BF16, kind='ExternalInput').ap()
    # partial y over this core's 8 heads (all 1024 out cols); host sums pairs
    ypart = nc.dram_tensor('ypart', [S, N_EMBD], BF16, kind='ExternalOutput').ap()

    def bcast_mid(t, n):
        # view [128, w] tile as [128, n, w] broadcasting over middle dim
        return bass.AP(tensor=t.tensor, offset=t.offset,
                       ap=[t.ap[0], [0, n], t.ap[-1]])

    def bcast_last(t, width):
        # view [128, n] tile as [128, n, width] broadcasting over last dim
        return bass.AP(tensor=t.tensor, offset=t.offset,
                       ap=[t.ap[0], t.ap[1], [0, width]])

    with tile.TileContext(nc) as tc:
        import contextlib
        ctx = contextlib.ExitStack()
        with ctx:
            const = ctx.enter_context(tc.tile_pool(name='const', bufs=1))
            persist = ctx.enter_context(tc.tile_pool(name='persist', bufs=1))

            ident = const.tile([128, 128], BF16)
            make_identity(nc, ident)
            epst = const.tile([128, 1], F32)
            nc.vector.memset(epst, RMS_EPS)
            # multiplicative causal mask for diagonal blocks: 1 where q >= k
            trimask = const.tile([128, 128], BF16)
            nc.gpsimd.memset(trimask, 1.0)
            nc.gpsimd.affine_select(
                out=trimask, in_=trimask, compare_op=ALU.is_ge,
                fill=0.0, base=0, pattern=[[1, 128]], channel_multiplier=-1)

            # persistent SBUF data
            xTs = [persist.tile([128, S], BF16, name=f'xTs{d}') for d in range(ND)]
            wq = [persist.tile([128, JW], BF16, name=f'wq{d}') for d in range(ND)]
            wo = [persist.tile([128, N_EMBD], BF16, name=f'wo{f}') for f in range(NHC)]
            cosb = [const.tile([128, HD // 2], BF16, name=f'cos{i}') for i in range(NB)]
            sinb = [const.tile([128, HD // 2], BF16, name=f'sin{i}') for i in range(NB)]
            qT = [persist.tile([128, S], BF16, name=f'qT{i}') for i in range(NHC)]
            kT = [persist.tile([128, S], BF16, name=f'kT{i}') for i in range(NHC)]
            vt = [persist.tile([128, HPC, 128], BF16, name=f'vt{i}') for i in range(NB)]
            attT = [persist.tile([128, S], BF16, name=f'attT{f}') for f in range(NHC)]
            # rnkt[nb][:, 0:8] = q-norm recip (with HD^-0.5), [:, 8:16] = k-norm recip
            rnkt = [persist.tile([128, N_HEAD], F32, name=f'rn{i}') for i in range(NB)]

            # prologue DMAs, interleaved so block-0 work can start early
            for d in range(ND):
                nc.sync.dma_start(out=xTs[d], in_=xT[d * 128:(d + 1) * 128])
                nc.sync.dma_start(out=wq[d], in_=wqkvT[d * 128:(d + 1) * 128])
            for nb in range(NB):
                nc.sync.dma_start(out=cosb[nb], in_=cosg[nb * 128:(nb + 1) * 128])
                nc.sync.dma_start(out=sinb[nb], in_=sing[nb * 128:(nb + 1) * 128])
            for f in range(NHC):
                nc.sync.dma_start(out=wo[f], in_=woT[f * 128:(f + 1) * 128])
            for nb in range(NB):
                nc.gpsimd.memset(vt[nb][:, :, 0:HD], 1.0)

            # ---- phase 1: QKV projection + rotary + rms + transposes ----
            # The q/k transposes are deferred until after all 8 blocks' QKV
            # matmuls so the PE sees one long uninterrupted stream (p-state).
            rott = [persist.tile([128, N_HEAD, HD], BF16, name=f'rot{i}')
                    for i in range(NB)]
            with tc.tile_pool(name='qkw', bufs=3) as qkw, \
                 tc.tile_pool(name='rotw', bufs=3) as rotw, \
                 tc.tile_pool(name='psq', bufs=2, space='PSUM') as psq, \
                 tc.tile_pool(name='ptr', bufs=2, space='PSUM') as ptr:
                for nb in range(NB):
                    rsl = slice(nb * 128, (nb + 1) * 128)
                    pq = psq.tile([128, 3 * HPC, HD], F32, tag='pq')
                    for d in range(ND):
                        for g in range(3):
                            nc.tensor.matmul(
                                pq[:, g * HPC:(g + 1) * HPC],
                                xTs[d][:, rsl],
                                wq[d][:, g * 512:(g + 1) * 512],
                                start=(d == 0), stop=(d == ND - 1))
                    # evacuate psum: q,k -> bf16 for rotary; v -> vt
                    qk = qkw.tile([128, N_HEAD, HD], BF16, tag='qk')
                    nc.scalar.copy(qk, pq[:, 0:N_HEAD])
                    nc.scalar.copy(vt[nb][:, :, HD:128], pq[:, N_HEAD:3 * HPC])

                    # rotary on q+k heads together (all bf16, 2x DVE)
                    cb = bcast_mid(cosb[nb], N_HEAD)
                    sb = bcast_mid(sinb[nb], N_HEAD)
                    x1 = qk[:, :, 0:32]
                    x2 = qk[:, :, 32:64]
                    rot = rott[nb]
                    scr = rotw.tile([128, N_HEAD, HD], BF16, tag='scr')
                    nc.vector.tensor_tensor(out=rot[:, :, 0:32], in0=x1, in1=cb, op=ALU.mult)
                    nc.vector.tensor_tensor(out=scr[:, :, 0:32], in0=x2, in1=sb, op=ALU.mult)
                    nc.vector.tensor_tensor(out=rot[:, :, 0:32], in0=rot[:, :, 0:32],
                                            in1=scr[:, :, 0:32], op=ALU.add)
                    nc.vector.tensor_tensor(out=rot[:, :, 32:64], in0=x2, in1=cb, op=ALU.mult)
                    nc.vector.tensor_tensor(out=scr[:, :, 32:64], in0=x1, in1=sb, op=ALU.mult)
                    nc.vector.tensor_tensor(out=rot[:, :, 32:64], in0=rot[:, :, 32:64],
                                            in1=scr[:, :, 32:64], op=ALU.subtract)

                    # rms: ms = sum(rot^2) over head dim; rn = 1/sqrt(ms/64+eps)
                    nc.vector.tensor_tensor(out=scr, in0=rot, in1=rot, op=ALU.mult)
                    ms = qkw.tile([128, N_HEAD], F32, tag='ms')
                    nc.vector.reduce_sum(out=ms, in_=scr, axis=mybir.AxisListType.X)
                    nc.scalar.activation(out=ms, in_=ms, func=ACT.Sqrt,
                                         bias=epst, scale=1.0 / HD)
                    nc.vector.reciprocal(out=rnkt[nb], in_=ms)
                    # fold HD^-0.5 into the q-side recips (bf16 for the mul)
                    rnq = qkw.tile([128, HPC], BF16, tag='rnq')
                    nc.scalar.mul(out=rnq, in_=rnkt[nb][:, 0:HPC], mul=HD ** -0.5)
                    nc.vector.tensor_tensor(out=rot[:, 0:HPC, :], in0=rot[:, 0:HPC, :],
                                            in1=bcast_last(rnq, HD), op=ALU.mult)

                # transpose q (normalized) and k (unnormalized) head-pairs
                for nb in range(NB):
                    rsl = slice(nb * 128, (nb + 1) * 128)
                    rot = rott[nb]
                    for hc in range(NHC):
                        pt = ptr.tile([128, 128], BF16, tag='pt')
                        nc.tensor.transpose(
                            pt, rot[:, 2 * hc:2 * hc + 2, :].rearrange("p a b -> p (a b)"),
                            ident)
                        nc.vector.tensor_copy(qT[hc][:, rsl], pt)
                        pt2 = ptr.tile([128, 128], BF16, tag='pt')
                        nc.tensor.transpose(
                            pt2, rot[:, HPC + 2 * hc:HPC + 2 * hc + 2, :].rearrange("p a b -> p (a b)"),
                            ident)
                        nc.scalar.copy(kT[hc][:, rsl], pt2)

            # ---- phase 2: attention (scores^T -> exp -> mask -> PV) ----
            with tc.tile_pool(name='estp', bufs=4) as estp, \
                 tc.tile_pool(name='pssc', bufs=2, space='PSUM') as pssc, \
                 tc.tile_pool(name='pspv', bufs=2, space='PSUM') as pspv:
                for h in range(HPC):
                    hc, h2 = h // 2, h % 2
                    dsl = slice(h2 * HD, (h2 + 1) * HD)
                    pv = pspv.tile([128, S], F32, tag='pv')

                    def chunks_of(kc):
                        c0 = kc * 128
                        return [(c0, 512), (512, S)] if c0 < 512 else [(c0, S)]

                    def scores(kc):
                        # returns est tile; emits scores matmuls + exp + mask
                        c0 = kc * 128
                        sct = pssc.tile([128, S], F32, tag='sct')
                        for a, b in chunks_of(kc):
                            nc.tensor.matmul(
                                sct[:, a:b],
                                kT[hc][dsl, c0:c0 + 128],
                                qT[hc][dsl, a:b],
                                start=True, stop=True)
                        est = estp.tile([128, S], BF16, tag='est')
                        nc.scalar.activation(out=est[:, c0:], in_=sct[:, c0:],
                                             func=ACT.Exp,
                                             scale=rnkt[kc][:, HPC + h:HPC + h + 1])
                        # zero the strictly-upper triangle of the diagonal block
                        # (gpsimd -- its queue is free now that there are no
                        # collectives, and it shortens the Act/DVE chains)
                        nc.gpsimd.affine_select(
                            out=est[:, c0:c0 + 128], in_=est[:, c0:c0 + 128],
                            compare_op=ALU.is_ge, fill=0.0, base=0,
                            pattern=[[1, 128]], channel_multiplier=-1)
                        return est

                    # software-pipelined emission: the PE queue is in-order, so
                    # keep scores(kc+1) AHEAD of pv(kc) -- pv waits on exp+mask
                    ests = {0: scores(0)}
                    for kc in range(NB):
                        if kc + 1 < NB:
                            ests[kc + 1] = scores(kc + 1)
                        est = ests.pop(kc)
                        for a, b in chunks_of(kc):
                            nc.tensor.matmul(
                                pv[:, a:b], vt[kc][:, h], est[:, a:b],
                                start=(kc == 0), stop=(kc == NB - 1),
                                skip_group_check=True)
                    # normalize by the ones-row denominators (pv rows 0:64 --
                    # ones-pad comes FIRST so the approx reciprocal sees a
                    # partition-0 input, which it requires)
                    rden = estp.tile([HD, S], F32, tag='rden')
                    nc.vector.reciprocal_approx_fast(out=rden, in_=pv[0:HD, :])
                    nc.vector.tensor_tensor(out=attT[hc][dsl, :], in0=pv[HD:128, :],
                                            in1=rden, op=ALU.mult)

            # ---- phase 3: output projection over all 16 heads ----
            with tc.tile_pool(name='yw', bufs=4) as yw, \
                 tc.tile_pool(name='psy', bufs=8, space='PSUM') as psy:
                # partial y over own 4 head-pairs, all 1024 out cols, emitted
                # slot-outer (head-pair outer) so row-blocks' matmuls for the
                # early head-pairs run while the last heads' attention finishes
                for og in range(2):
                    osl = slice(og * 512, (og + 1) * 512)
                    py = [psy.tile([128, 512], F32, tag='py', name=f'py{og}_{qt}')
                          for qt in range(NB)]
                    for f in range(NHC - 1):
                        for qt in range(NB):
                            nc.tensor.matmul(
                                py[qt], attT[f][:, qt * 128:(qt + 1) * 128],
                                wo[f][:, osl],
                                start=(f == 0), stop=False)
                    for qt in range(NB):
                        # last head-pair + evac interleaved per row-block so
                        # the next og wave's psum banks free up progressively
                        nc.tensor.matmul(
                            py[qt], attT[NHC - 1][:, qt * 128:(qt + 1) * 128],
                            wo[NHC - 1][:, osl], start=False, stop=True)
                        ys = yw.tile([128, 512], BF16, tag='ys')
                        if qt % 2 == 0:
                            nc.vector.tensor_copy(ys, py[qt])
                        else:
                            nc.scalar.copy(ys, py[qt])
                        nc.sync.dma_start(
                            out=ypart[qt * 128:(qt + 1) * 128, osl], in_=ys)

    nc.compile()
    return nc


def _get_nc():
    if 'nc' not in _cached:
        _cached['nc'] = _build()
    return _cached['nc']


def kernel(x, Wqkv, Wo, cos_cache, sin_cache, cu_seqlens, position_ids,
           max_seqlen, **_ignored):
    from concourse.bass_utils import run_bass_kernel_spmd
    import ml_dtypes

    bf16 = ml_dtypes.bfloat16
    x = np.asarray(x, dtype=np.float32)
    Wqkv = np.asarray(Wqkv, dtype=np.float32)
    Wo = np.asarray(Wo, dtype=np.float32)
    cos_cache = np.asarray(cos_cache, dtype=np.float32)
    sin_cache = np.asarray(sin_cache, dtype=np.float32)
    position_ids = np.asarray(position_ids)

    nc = _get_nc()
    in_maps = []
    for c in range(NCORES):
        b, hh = c // 2, c % 2
        rows = slice(b * S, (b + 1) * S)
        qsl = slice(hh * HPC * HD, (hh + 1) * HPC * HD)
        ksl = slice(N_EMBD + hh * HPC * HD, N_EMBD + (hh + 1) * HPC * HD)
        vsl = slice(2 * N_EMBD + hh * HPC * HD, 2 * N_EMBD + (hh + 1) * HPC * HD)
        wqkvT_c = np.concatenate(
            [Wqkv[qsl], Wqkv[ksl], Wqkv[vsl]], axis=0).T
        # own heads' contraction rows (Wo columns), all 1024 out columns
        woT_c = Wo[:, hh * HPC * HD:(hh + 1) * HPC * HD].T
        pos = position_ids[rows]
        in_maps.append({
            'xT': np.ascontiguousarray(x[rows].T).astype(bf16),
            'wqkvT': np.ascontiguousarray(wqkvT_c).astype(bf16),
            'woT': np.ascontiguousarray(woT_c).astype(bf16),
            'cosg': np.ascontiguousarray(cos_cache[pos]).astype(bf16),
            'sing': np.ascontiguousarray(sin_cache[pos]).astype(bf16),
        })

    r = run_bass_kernel_spmd(nc, in_maps, list(range(NCORES)))
    out = np.empty((N, N_EMBD), dtype=np.float32)
    for b in range(B):
        rows = slice(b * S, (b + 1) * S)
        out[rows] = (np.asarray(r.results[2 * b]['ypart']).astype(np.float32) +
                     np.asarray(r.results[2 * b + 1]['ypart']).astype(np.float32))
    _cached['last_results'] = r
    return out


# revision 58
# speedup vs baseline: 1.1022x; 1.1022x over previous
"""Causal varlen self-attention (packed equal-length sequences) on 8 trn2 cores.

Sharding: 4 sequences x 2 head-groups. Core c handles sequence b = c//2 and
heads hh*8..hh*8+8 (hh = c%2). Each core computes the QKV projection of its
sequence restricted to its 8 heads, rotary+RMSNorm, causal attention for all
1024 rows over its heads, exchanges bf16 attention outputs with its pair
partner via AllGather, and computes the final output projection for its
512-wide column slice of y (even core: out cols 0..512, odd: 512..1024) over
the full 1024-feature contraction. The host assembles y column-wise -- the
program is SPMD-symmetric with no all-reduce.

All matmul inputs are bf16 (f32 PSUM accumulation); the host pre-transposes x
and pre-converts weights. Scores are computed in a transposed layout
[kpos, q] with causal column-trimming (matmuls/exp/PV only touch q >= kc*128);
within-diagonal-block masking zeroes est's upper triangle on the gpsimd
engine after exp. The k-side RMS norm is folded into exp's per-partition
scale; softmax denominators come from 64 ones-columns appended to V and one
vector divide per head normalizes the attention output.
"""
import numpy as np

N_EMBD = 1024
N_HEAD = 16
HD = 64
S = 1024
B = 4
N = B * S
NCORES = 8
HPC = 8            # heads per core
NHC = HPC // 2     # head-pair chunks per core
NB = S // 128      # row blocks per sequence
ND = N_EMBD // 128  # contraction chunks
JW = 3 * HPC * HD  # qkv feature width per core (1536)
OW = N_EMBD // 2   # output columns per core (512)
RMS_EPS = 1.1920929e-07

_cached = {}


def _build():
    import concourse.bacc as bacc
    import concourse.mybir as mybir
    import concourse.tile as tile
    import concourse.bass as bass
    from concourse.masks import make_identity

    F32 = mybir.dt.float32
    BF16 = mybir.dt.bfloat16
    ALU = mybir.AluOpType
    ACT = mybir.ActivationFunctionType

    nc = bacc.Bacc('TRN2', target_bir_lowering=False, debug=False,
                   num_devices=NCORES)
    xT = nc.dram_tensor('xT', [N_EMBD, S], BF16, kind='ExternalInput').ap()
    wqkvT = nc.dram_tensor('wqkvT', [N_EMBD, JW], BF16, kind='ExternalInput').ap()
    woT = nc.dram_tensor('woT', [NHC * 128, N_EMBD], BF16, kind='ExternalInput').ap()
    cosg = nc.dram_tensor('cosg', [S, HD // 2], BF16, kind='ExternalInput').ap()
    sing = nc.dram_tensor('sing', [S, HD // 2], BF16, kind='ExternalInput').ap()
    # partial y over this core's 8 heads (all 1024 out cols); host sums pairs
    ypart = nc.dram_tensor('ypart', [S, N_EMBD], BF16, kind='ExternalOutput').ap()

    def bcast_mid(t, n):
        # view [128, w] tile as [128, n, w] broadcasting over middle dim
        return bass.AP(tensor=t.tensor, offset=t.offset,
                       ap=[t.ap[0], [0, n], t.ap[-1]])

    def bcast_last(t, width):
        # view [128, n] tile as [128, n, width] broadcasting over last dim
        return bass.AP(tensor=t.tensor, offset=t.offset,
                       ap=[t.ap[0], t.ap[1], [0, width]])

    with tile.TileContext(nc) as tc:
        import contextlib
        ctx = contextlib.ExitStack()
        with ctx:
            const = ctx.enter_context(tc.tile_pool(name='const', bufs=1))
            persist = ctx.enter_context(tc.tile_pool(name='persist', bufs=1))

            ident = const.tile([128, 128], BF16)
            make_identity(nc, ident)
            epst = const.tile([128, 1], F32)
            nc.vector.memset(epst, RMS_EPS)
            # multiplicative causal mask for diagonal blocks: 1 where q >= k
            trimask = const.tile([128, 128], BF16)
            nc.gpsimd.memset(trimask, 1.0)
            nc.gpsimd.affine_select(
                out=trimask, in_=trimask, compare_op=ALU.is_ge,
                fill=0.0, base=0, pattern=[[1, 128]], channel_multiplier=-1)

            # persistent SBUF data
            xTs = [persist.tile([128, S], BF16, name=f'xTs{d}') for d in range(ND)]
            wq = [persist.tile([128, JW], BF16, name=f'wq{d}') for d in range(ND)]
            wo = [persist.tile([128, N_EMBD], BF16, name=f'wo{f}') for f in range(NHC)]
            cosb = [const.tile([128, HD // 2], BF16, name=f'cos{i}') for i in range(NB)]
            sinb = [const.tile([128, HD // 2], BF16, name=f'sin{i}') for i in range(NB)]
            qT = [persist.tile([128, S], BF16, name=f'qT{i}') for i in range(NHC)]
            kT = [persist.tile([128, S], BF16, name=f'kT{i}') for i in range(NHC)]
            vt = [persist.tile([128, HPC, 128], BF16, name=f'vt{i}') for i in range(NB)]
            attT = [persist.tile([128, S], BF16, name=f'attT{f}') for f in range(NHC)]
            # rnkt[nb][:, 0:8] = q-norm recip (with HD^-0.5), [:, 8:16] = k-norm recip
            rnkt = [persist.tile([128, N_HEAD], F32, name=f'rn{i}') for i in range(NB)]

            # prologue DMAs, interleaved so block-0 work can start early
            for d in range(ND):
                nc.sync.dma_start(out=xTs[d], in_=xT[d * 128:(d + 1) * 128])
                nc.sync.dma_start(out=wq[d], in_=wqkvT[d * 128:(d + 1) * 128])
            for nb in range(NB):
                nc.sync.dma_start(out=cosb[nb], in_=cosg[nb * 128:(nb + 1) * 128])
                nc.sync.dma_start(out=sinb[nb], in_=sing[nb * 128:(nb + 1) * 128])
            for f in range(NHC):
                nc.sync.dma_start(out=wo[f], in_=woT[f * 128:(f + 1) * 128])
            for nb in range(NB):
                nc.gpsimd.memset(vt[nb][:, :, 0:HD], 1.0)

            # ---- phase 1: QKV projection + rotary + rms + transposes ----
            # The q/k transposes are deferred until after all 8 blocks' QKV
            # matmuls so the PE sees one long uninterrupted stream (p-state).
            rott = [persist.tile([128, N_HEAD, HD], BF16, name=f'rot{i}')
                    for i in range(NB)]
            with tc.tile_pool(name='qkw', bufs=3) as qkw, \
                 tc.tile_pool(name='rotw', bufs=3) as rotw, \
                 tc.tile_pool(name='psq', bufs=2, space='PSUM') as psq, \
                 tc.tile_pool(name='ptr', bufs=2, space='PSUM') as ptr:
                for nb in range(NB):
                    rsl = slice(nb * 128, (nb + 1) * 128)
                    pq = psq.tile([128, 3 * HPC, HD], F32, tag='pq')
                    for d in range(ND):
                        for g in range(3):
                            nc.tensor.matmul(
                                pq[:, g * HPC:(g + 1) * HPC],
                                xTs[d][:, rsl],
                                wq[d][:, g * 512:(g + 1) * 512],
                                start=(d == 0), stop=(d == ND - 1))
                    # evacuate psum: q,k -> bf16 for rotary; v -> vt
                    qk = qkw.tile([128, N_HEAD, HD], BF16, tag='qk')
                    nc.scalar.copy(qk, pq[:, 0:N_HEAD])
                    nc.scalar.copy(vt[nb][:, :, HD:128], pq[:, N_HEAD:3 * HPC])

                    # rotary on q+k heads together (all bf16, 2x DVE)
                    cb = bcast_mid(cosb[nb], N_HEAD)
                    sb = bcast_mid(sinb[nb], N_HEAD)
                    x1 = qk[:, :, 0:32]
                    x2 = qk[:, :, 32:64]
                    rot = rott[nb]
                    scr = rotw.tile([128, N_HEAD, HD], BF16, tag='scr')
                    nc.vector.tensor_tensor(out=rot[:, :, 0:32], in0=x1, in1=cb, op=ALU.mult)
                    nc.vector.tensor_tensor(out=scr[:, :, 0:32], in0=x2, in1=sb, op=ALU.mult)
                    nc.vector.tensor_tensor(out=rot[:, :, 0:32], in0=rot[:, :, 0:32],
                                            in1=scr[:, :, 0:32], op=ALU.add)
                    nc.vector.tensor_tensor(out=rot[:, :, 32:64], in0=x2, in1=cb, op=ALU.mult)
                    nc.vector.tensor_tensor(out=scr[:, :, 32:64], in0=x1, in1=sb, op=ALU.mult)
                    nc.vector.tensor_tensor(out=rot[:, :, 32:64], in0=rot[:, :, 32:64],
                                            in1=scr[:, :, 32:64], op=ALU.subtract)

                    # rms: ms = sum(rot^2) over head dim; rn = 1/sqrt(ms/64+eps)
                    nc.vector.tensor_tensor(out=scr, in0=rot, in1=rot, op=ALU.mult)
                    ms = qkw.tile([128, N_HEAD], F32, tag='ms')
                    nc.vector.reduce_sum(out=ms, in_=scr, axis=mybir.AxisListType.X)
                    nc.scalar.activation(out=ms, in_=ms, func=ACT.Sqrt,
                                         bias=epst, scale=1.0 / HD)
                    nc.vector.reciprocal(out=rnkt[nb], in_=ms)
                    # fold HD^-0.5 into the q-side recips (bf16 for the mul)
                    rnq = qkw.tile([128, HPC], BF16, tag='rnq')
                    nc.scalar.mul(out=rnq, in_=rnkt[nb][:, 0:HPC], mul=HD ** -0.5)
                    nc.vector.tensor_tensor(out=rot[:, 0:HPC, :], in0=rot[:, 0:HPC, :],
                                            in1=bcast_last(rnq, HD), op=ALU.mult)

                # transpose q (normalized) and k (unnormalized) head-pairs
                for nb in range(NB):
                    rsl = slice(nb * 128, (nb + 1) * 128)
                    rot = rott[nb]
                    for hc in range(NHC):
                        pt = ptr.tile([128, 128], BF16, tag='pt')
                        nc.tensor.transpose(
                            pt, rot[:, 2 * hc:2 * hc + 2, :].rearrange("p a b -> p (a b)"),
                            ident)
                        nc.vector.tensor_copy(qT[hc][:, rsl], pt)
                        pt2 = ptr.tile([128, 128], BF16, tag='pt')
                        nc.tensor.transpose(
                            pt2, rot[:, HPC + 2 * hc:HPC + 2 * hc + 2, :].rearrange("p a b -> p (a b)"),
                            ident)
                        nc.scalar.copy(kT[hc][:, rsl], pt2)

            # ---- phase 2: attention (scores^T -> exp -> mask -> PV) ----
            with tc.tile_pool(name='estp', bufs=4) as estp, \
                 tc.tile_pool(name='pssc', bufs=2, space='PSUM') as pssc, \
                 tc.tile_pool(name='pspv', bufs=2, space='PSUM') as pspv:
                for h in range(HPC):
                    hc, h2 = h // 2, h % 2
                    dsl = slice(h2 * HD, (h2 + 1) * HD)
                    pv = pspv.tile([128, S], F32, tag='pv')

                    def chunks_of(kc):
                        c0 = kc * 128
                        return [(c0, 512), (512, S)] if c0 < 512 else [(c0, S)]

                    def scores(kc):
                        # returns est tile; emits scores matmuls + exp + mask
                        c0 = kc * 128
                        sct = pssc.tile([128, S], F32, tag='sct')
                        for a, b in chunks_of(kc):
                            nc.tensor.matmul(
                                sct[:, a:b],
                                kT[hc][dsl, c0:c0 + 128],
                                qT[hc][dsl, a:b],
                                start=True, stop=True)
                        est = estp.tile([128, S], BF16, tag='est')
                        nc.scalar.activation(out=est[:, c0:], in_=sct[:, c0:],
                                             func=ACT.Exp,
                                             scale=rnkt[kc][:, HPC + h:HPC + h + 1])
                        # zero the strictly-upper triangle of the diagonal block
                        nc.vector.tensor_tensor(
                            out=est[:, c0:c0 + 128], in0=est[:, c0:c0 + 128],
                            in1=trimask, op=ALU.mult)
                        return est

                    # software-pipelined emission: the PE queue is in-order, so
                    # keep scores(kc+1) AHEAD of pv(kc) -- pv waits on exp+mask
                    ests = {0: scores(0)}
                    for kc in range(NB):
                        if kc + 1 < NB:
                            ests[kc + 1] = scores(kc + 1)
                        est = ests.pop(kc)
                        for a, b in chunks_of(kc):
                            nc.tensor.matmul(
                                pv[:, a:b], vt[kc][:, h], est[:, a:b],
                                start=(kc == 0), stop=(kc == NB - 1),
                                skip_group_check=True)
                    # normalize by the ones-row denominators (pv rows 0:64 --
                    # ones-pad comes FIRST so the approx reciprocal sees a
                    # partition-0 input, which it requires)
                    rden = estp.tile([HD, S], F32, tag='rden')
                    nc.vector.reciprocal_approx_fast(out=rden, in_=pv[0:HD, :])
                    nc.vector.tensor_tensor(out=attT[hc][dsl, :], in0=pv[HD:128, :],
                                            in1=rden, op=ALU.mult)

            # ---- phase 3: output projection over all 16 heads ----
            with tc.tile_pool(name='yw', bufs=4) as yw, \
                 tc.tile_pool(name='psy', bufs=8, space='PSUM') as psy:
                # partial y over own 4 head-pairs, all 1024 out cols, emitted
                # slot-outer (head-pair outer) so row-blocks' matmuls for the
                # early head-pairs run while the last heads' attention finishes
                for og in range(2):
                    osl = slice(og * 512, (og + 1) * 512)
                    py = [psy.tile([128, 512], F32, tag='py', name=f'py{og}_{qt}')
                          for qt in range(NB)]
                    for f in range(NHC - 1):
                        for qt in range(NB):
                            nc.tensor.matmul(
                                py[qt], attT[f][:, qt * 128:(qt + 1) * 128],
                                wo[f][:, osl],
                                start=(f == 0), stop=False)
                    for qt in range(NB):
                        # last head-pair + evac interleaved per row-block so
                        # the next og wave's psum banks free up progressively
                        nc.tensor.matmul(
                            py[qt], attT[NHC - 1][:, qt * 128:(qt + 1) * 128],
                            wo[NHC - 1][:, osl], start=False, stop=True)
                        ys = yw.tile([128, 512], BF16, tag='ys')
                        if qt % 2 == 0:
                            nc.vector.tensor_copy(ys, py[qt])
                        else:
                            nc.scalar.copy(ys, py[qt])
                        nc.sync.dma_start(
                            out=ypart[qt * 128:(qt + 1) * 128, osl], in_=ys)

    nc.compile()
    return nc


def _get_nc():
    if 'nc' not in _cached:
        _cached['nc'] = _build()
    return _cached['nc']


def kernel(x, Wqkv, Wo, cos_cache, sin_cache, cu_seqlens, position_ids,
           max_seqlen, **_ignored):
    from concourse.bass_utils import run_bass_kernel_spmd
    import ml_dtypes

    bf16 = ml_dtypes.bfloat16
    x = np.asarray(x, dtype=np.float32)
    Wqkv = np.asarray(Wqkv, dtype=np.float32)
    Wo = np.asarray(Wo, dtype=np.float32)
    cos_cache = np.asarray(cos_cache, dtype=np.float32)
    sin_cache = np.asarray(sin_cache, dtype=np.float32)
    position_ids = np.asarray(position_ids)

    nc = _get_nc()
    in_maps = []
    for c in range(NCORES):
        b, hh = c // 2, c % 2
        rows = slice(b * S, (b + 1) * S)
        qsl = slice(hh * HPC * HD, (hh + 1) * HPC * HD)
        ksl = slice(N_EMBD + hh * HPC * HD, N_EMBD + (hh + 1) * HPC * HD)
        vsl = slice(2 * N_EMBD + hh * HPC * HD, 2 * N_EMBD + (hh + 1) * HPC * HD)
        wqkvT_c = np.concatenate(
            [Wqkv[qsl], Wqkv[ksl], Wqkv[vsl]], axis=0).T
        # own heads' contraction rows (Wo columns), all 1024 out columns
        woT_c = Wo[:, hh * HPC * HD:(hh + 1) * HPC * HD].T
        pos = position_ids[rows]
        in_maps.append({
            'xT': np.ascontiguousarray(x[rows].T).astype(bf16),
            'wqkvT': np.ascontiguousarray(wqkvT_c).astype(bf16),
            'woT': np.ascontiguousarray(woT_c).astype(bf16),
            'cosg': np.ascontiguousarray(cos_cache[pos]).astype(bf16),
            'sing': np.ascontiguousarray(sin_cache[pos]).astype(bf16),
        })

    r = run_bass_kernel_spmd(nc, in_maps, list(range(NCORES)))
    out = np.empty((N, N_EMBD), dtype=np.float32)
    for b in range(B):
        rows = slice(b * S, (b + 1) * S)
        out[rows] = (np.asarray(r.results[2 * b]['ypart']).astype(np.float32) +
                     np.asarray(r.results[2 * b + 1]['ypart']).astype(np.float32))
    _cached['last_results'] = r
    return out


# revision 60
# speedup vs baseline: 1.1436x; 1.0376x over previous
"""Causal varlen self-attention (packed equal-length sequences) on 8 trn2 cores.

Sharding: 4 sequences x 2 head-groups. Core c handles sequence b = c//2 and
heads hh*8..hh*8+8 (hh = c%2). Each core computes the QKV projection of its
sequence restricted to its 8 heads, rotary+RMSNorm, causal attention for all
1024 rows over its heads, exchanges bf16 attention outputs with its pair
partner via AllGather, and computes the final output projection for its
512-wide column slice of y (even core: out cols 0..512, odd: 512..1024) over
the full 1024-feature contraction. The host assembles y column-wise -- the
program is SPMD-symmetric with no all-reduce.

All matmul inputs are bf16 (f32 PSUM accumulation); the host pre-transposes x
and pre-converts weights. Scores are computed in a transposed layout
[kpos, q] with causal column-trimming (matmuls/exp/PV only touch q >= kc*128);
within-diagonal-block masking zeroes est's upper triangle on the gpsimd
engine after exp. The k-side RMS norm is folded into exp's per-partition
scale; softmax denominators come from 64 ones-columns appended to V and one
vector divide per head normalizes the attention output.
"""
import numpy as np

N_EMBD = 1024
N_HEAD = 16
HD = 64
S = 1024
B = 4
N = B * S
NCORES = 8
HPC = 8            # heads per core
NHC = HPC // 2     # head-pair chunks per core
NB = S // 128      # row blocks per sequence
ND = N_EMBD // 128  # contraction chunks
JW = 3 * HPC * HD  # qkv feature width per core (1536)
OW = N_EMBD // 2   # output columns per core (512)
RMS_EPS = 1.1920929e-07

_cached = {}


def _build():
    import concourse.bacc as bacc
    import concourse.mybir as mybir
    import concourse.tile as tile
    import concourse.bass as bass
    from concourse.masks import make_identity

    F32 = mybir.dt.float32
    BF16 = mybir.dt.bfloat16
    ALU = mybir.AluOpType
    ACT = mybir.ActivationFunctionType

    nc = bacc.Bacc('TRN2', target_bir_lowering=False, debug=False,
                   num_devices=NCORES)
    xT = nc.dram_tensor('xT', [N_EMBD, S], BF16, kind='ExternalInput').ap()
    wqkvT = nc.dram_tensor('wqkvT', [N_EMBD, JW], BF16, kind='ExternalInput').ap()
    woT = nc.dram_tensor('woT', [NHC * 128, N_EMBD], BF16, kind='ExternalInput').ap()
    cosg = nc.dram_tensor('cosg', [S, HD // 2], BF16, kind='ExternalInput').ap()
    sing = nc.dram_tensor('sing', [S, HD // 2], BF16, kind='ExternalInput').ap()
    # partial y over this core's 8 heads (all 1024 out cols); host sums pairs
    ypart = nc.dram_tensor('ypart', [S, N_EMBD], BF16, kind='ExternalOutput').ap()

    def bcast_mid(t, n):
        # view [128, w] tile as [128, n, w] broadcasting over middle dim
        return bass.AP(tensor=t.tensor, offset=t.offset,
                       ap=[t.ap[0], [0, n], t.ap[-1]])

    def bcast_last(t, width):
        # view [128, n] tile as [128, n, width] broadcasting over last dim
        return bass.AP(tensor=t.tensor, offset=t.offset,
                       ap=[t.ap[0], t.ap[1], [0, width]])

    with tile.TileContext(nc) as tc:
        import contextlib
        ctx = contextlib.ExitStack()
        with ctx:
            const = ctx.enter_context(tc.tile_pool(name='const', bufs=1))
            persist = ctx.enter_context(tc.tile_pool(name='persist', bufs=1))

            ident = const.tile([128, 128], BF16)
            make_identity(nc, ident)
            epst = const.tile([128, 1], F32)
            nc.vector.memset(epst, RMS_EPS)
            # multiplicative causal mask for diagonal blocks: 1 where q >= k
            trimask = const.tile([128, 128], BF16)
            nc.gpsimd.memset(trimask, 1.0)
            nc.gpsimd.affine_select(
                out=trimask, in_=trimask, compare_op=ALU.is_ge,
                fill=0.0, base=0, pattern=[[1, 128]], channel_multiplier=-1)

            # persistent SBUF data
            xTs = [persist.tile([128, S], BF16, name=f'xTs{d}') for d in range(ND)]
            wq = [persist.tile([128, JW], BF16, name=f'wq{d}') for d in range(ND)]
            wo = [persist.tile([128, N_EMBD], BF16, name=f'wo{f}') for f in range(NHC)]
            cosb = [const.tile([128, HD // 2], BF16, name=f'cos{i}') for i in range(NB)]
            sinb = [const.tile([128, HD // 2], BF16, name=f'sin{i}') for i in range(NB)]
            qT = [persist.tile([128, S], BF16, name=f'qT{i}') for i in range(NHC)]
            kT = [persist.tile([128, S], BF16, name=f'kT{i}') for i in range(NHC)]
            vt = [persist.tile([128, HPC, 128], BF16, name=f'vt{i}') for i in range(NB)]
            attT = [persist.tile([128, S], BF16, name=f'attT{f}') for f in range(NHC)]
            # rnkt[nb][:, 0:8] = q-norm recip (with HD^-0.5), [:, 8:16] = k-norm recip
            rnkt = [persist.tile([128, N_HEAD], F32, name=f'rn{i}') for i in range(NB)]

            # prologue DMAs, interleaved so block-0 work can start early
            for d in range(ND):
                nc.sync.dma_start(out=xTs[d], in_=xT[d * 128:(d + 1) * 128])
                nc.sync.dma_start(out=wq[d], in_=wqkvT[d * 128:(d + 1) * 128])
            for nb in range(NB):
                nc.sync.dma_start(out=cosb[nb], in_=cosg[nb * 128:(nb + 1) * 128])
                nc.sync.dma_start(out=sinb[nb], in_=sing[nb * 128:(nb + 1) * 128])
            for f in range(NHC):
                nc.sync.dma_start(out=wo[f], in_=woT[f * 128:(f + 1) * 128])
            for nb in range(NB):
                nc.gpsimd.memset(vt[nb][:, :, 0:HD], 1.0)

            # PE warm-up: ~4us of dummy back-to-back matmuls on the identity
            # tile so the tensor engine's p-state ramps to peak clock before
            # the first real QKV matmul (the DVFS ramp needs ~3us of
            # continuous execution; the QKV stream otherwise pays it)
            with tc.tile_pool(name='warm', bufs=1, space='PSUM') as warm:
                wt = warm.tile([128, 128], BF16, tag='wt')
                for _ in range(40):
                    nc.tensor.transpose(wt, ident, ident)

            # ---- phase 1: QKV projection + rotary + rms + transposes ----
            # The q/k transposes are deferred until after all 8 blocks' QKV
            # matmuls so the PE sees one long uninterrupted stream (p-state).
            rott = [persist.tile([128, N_HEAD, HD], BF16, name=f'rot{i}')
                    for i in range(NB)]
            with tc.tile_pool(name='qkw', bufs=3) as qkw, \
                 tc.tile_pool(name='rotw', bufs=3) as rotw, \
                 tc.tile_pool(name='psq', bufs=2, space='PSUM') as psq, \
                 tc.tile_pool(name='ptr', bufs=2, space='PSUM') as ptr:
                for nb in range(NB):
                    rsl = slice(nb * 128, (nb + 1) * 128)
                    pq = psq.tile([128, 3 * HPC, HD], F32, tag='pq')
                    for d in range(ND):
                        for g in range(3):
                            nc.tensor.matmul(
                                pq[:, g * HPC:(g + 1) * HPC],
                                xTs[d][:, rsl],
                                wq[d][:, g * 512:(g + 1) * 512],
                                start=(d == 0), stop=(d == ND - 1))
                    # evacuate psum: q,k -> bf16 for rotary; v -> vt
                    qk = qkw.tile([128, N_HEAD, HD], BF16, tag='qk')
                    nc.scalar.copy(qk, pq[:, 0:N_HEAD])
                    nc.scalar.copy(vt[nb][:, :, HD:128], pq[:, N_HEAD:3 * HPC])

                    # rotary on q+k heads together (all bf16, 2x DVE)
                    cb = bcast_mid(cosb[nb], N_HEAD)
                    sb = bcast_mid(sinb[nb], N_HEAD)
                    x1 = qk[:, :, 0:32]
                    x2 = qk[:, :, 32:64]
                    rot = rott[nb]
                    scr = rotw.tile([128, N_HEAD, HD], BF16, tag='scr')
                    nc.vector.tensor_tensor(out=rot[:, :, 0:32], in0=x1, in1=cb, op=ALU.mult)
                    nc.vector.tensor_tensor(out=scr[:, :, 0:32], in0=x2, in1=sb, op=ALU.mult)
                    nc.vector.tensor_tensor(out=rot[:, :, 0:32], in0=rot[:, :, 0:32],
                                            in1=scr[:, :, 0:32], op=ALU.add)
                    nc.vector.tensor_tensor(out=rot[:, :, 32:64], in0=x2, in1=cb, op=ALU.mult)
                    nc.vector.tensor_tensor(out=scr[:, :, 32:64], in0=x1, in1=sb, op=ALU.mult)
                    nc.vector.tensor_tensor(out=rot[:, :, 32:64], in0=rot[:, :, 32:64],
                                            in1=scr[:, :, 32:64], op=ALU.subtract)

                    # rms: ms = sum(rot^2) over head dim; rn = 1/sqrt(ms/64+eps)
                    nc.vector.tensor_tensor(out=scr, in0=rot, in1=rot, op=ALU.mult)
                    ms = qkw.tile([128, N_HEAD], F32, tag='ms')
                    nc.vector.reduce_sum(out=ms, in_=scr, axis=mybir.AxisListType.X)
                    nc.scalar.activation(out=ms, in_=ms, func=ACT.Sqrt,
                                         bias=epst, scale=1.0 / HD)
                    nc.vector.reciprocal(out=rnkt[nb], in_=ms)
                    # fold HD^-0.5 into the q-side recips (bf16 for the mul)
                    rnq = qkw.tile([128, HPC], BF16, tag='rnq')
                    nc.scalar.mul(out=rnq, in_=rnkt[nb][:, 0:HPC], mul=HD ** -0.5)
                    nc.vector.tensor_tensor(out=rot[:, 0:HPC, :], in0=rot[:, 0:HPC, :],
                                            in1=bcast_last(rnq, HD), op=ALU.mult)

                # transpose q (normalized) and k (unnormalized) head-pairs
                for nb in range(NB):
                    rsl = slice(nb * 128, (nb + 1) * 128)
                    rot = rott[nb]
                    for hc in range(NHC):
                        pt = ptr.tile([128, 128], BF16, tag='pt')
                        nc.tensor.transpose(
                            pt, rot[:, 2 * hc:2 * hc + 2, :].rearrange("p a b -> p (a b)"),
                            ident)
                        nc.vector.tensor_copy(qT[hc][:, rsl], pt)
                        pt2 = ptr.tile([128, 128], BF16, tag='pt')
                        nc.tensor.transpose(
                            pt2, rot[:, HPC + 2 * hc:HPC + 2 * hc + 2, :].rearrange("p a b -> p (a b)"),
                            ident)
                        nc.scalar.copy(kT[hc][:, rsl], pt2)

            # ---- phase 2: attention (scores^T -> exp -> mask -> PV) ----
            with tc.tile_pool(name='estp', bufs=4) as estp, \
                 tc.tile_pool(name='pssc', bufs=2, space='PSUM') as pssc, \
                 tc.tile_pool(name='pspv', bufs=2, space='PSUM') as pspv:
                for h in range(HPC):
                    hc, h2 = h // 2, h % 2
                    dsl = slice(h2 * HD, (h2 + 1) * HD)
                    pv = pspv.tile([128, S], F32, tag='pv')

                    def chunks_of(kc):
                        c0 = kc * 128
                        return [(c0, 512), (512, S)] if c0 < 512 else [(c0, S)]

                    def scores(kc):
                        # returns est tile; emits scores matmuls + exp + mask
                        c0 = kc * 128
                        sct = pssc.tile([128, S], F32, tag='sct')
                        for a, b in chunks_of(kc):
                            nc.tensor.matmul(
                                sct[:, a:b],
                                kT[hc][dsl, c0:c0 + 128],
                                qT[hc][dsl, a:b],
                                start=True, stop=True)
                        est = estp.tile([128, S], BF16, tag='est')
                        nc.scalar.activation(out=est[:, c0:], in_=sct[:, c0:],
                                             func=ACT.Exp,
                                             scale=rnkt[kc][:, HPC + h:HPC + h + 1])
                        # zero the strictly-upper triangle of the diagonal block
                        nc.vector.tensor_tensor(
                            out=est[:, c0:c0 + 128], in0=est[:, c0:c0 + 128],
                            in1=trimask, op=ALU.mult)
                        return est

                    # software-pipelined emission: the PE queue is in-order, so
                    # keep scores(kc+1) AHEAD of pv(kc) -- pv waits on exp+mask
                    ests = {0: scores(0)}
                    for kc in range(NB):
                        if kc + 1 < NB:
                            ests[kc + 1] = scores(kc + 1)
                        est = ests.pop(kc)
                        for a, b in chunks_of(kc):
                            nc.tensor.matmul(
                                pv[:, a:b], vt[kc][:, h], est[:, a:b],
                                start=(kc == 0), stop=(kc == NB - 1),
                                skip_group_check=True)
                    # normalize by the ones-row denominators (pv rows 0:64 --
                    # ones-pad comes FIRST so the approx reciprocal sees a
                    # partition-0 input, which it requires)
                    rden = estp.tile([HD, S], F32, tag='rden')
                    nc.vector.reciprocal_approx_fast(out=rden, in_=pv[0:HD, :])
                    nc.vector.tensor_tensor(out=attT[hc][dsl, :], in0=pv[HD:128, :],
                                            in1=rden, op=ALU.mult)

            # ---- phase 3: output projection over all 16 heads ----
            with tc.tile_pool(name='yw', bufs=4) as yw, \
                 tc.tile_pool(name='psy', bufs=8, space='PSUM') as psy:
                # partial y over own 4 head-pairs, all 1024 out cols, emitted
                # slot-outer (head-pair outer) so row-blocks' matmuls for the
                # early head-pairs run while the last heads' attention finishes
                for og in range(2):
                    osl = slice(og * 512, (og + 1) * 512)
                    py = [psy.tile([128, 512], F32, tag='py', name=f'py{og}_{qt}')
                          for qt in range(NB)]
                    for f in range(NHC):
                        for qt in range(NB):
                            nc.tensor.matmul(
                                py[qt], attT[f][:, qt * 128:(qt + 1) * 128],
                                wo[f][:, osl],
                                start=(f == 0), stop=(f == NHC - 1))
                    for qt in range(NB):
                        ys = yw.tile([128, 512], BF16, tag='ys')
                        if qt % 2 == 0:
                            nc.vector.tensor_copy(ys, py[qt])
                        else:
                            nc.scalar.copy(ys, py[qt])
                        nc.sync.dma_start(
                            out=ypart[qt * 128:(qt + 1) * 128, osl], in_=ys)

    nc.compile()
    return nc


def _get_nc():
    if 'nc' not in _cached:
        _cached['nc'] = _build()
    return _cached['nc']


def kernel(x, Wqkv, Wo, cos_cache, sin_cache, cu_seqlens, position_ids,
           max_seqlen, **_ignored):
    from concourse.bass_utils import run_bass_kernel_spmd
    import ml_dtypes

    bf16 = ml_dtypes.bfloat16
    x = np.asarray(x, dtype=np.float32)
    Wqkv = np.asarray(Wqkv, dtype=np.float32)
    Wo = np.asarray(Wo, dtype=np.float32)
    cos_cache = np.asarray(cos_cache, dtype=np.float32)
    sin_cache = np.asarray(sin_cache, dtype=np.float32)
    position_ids = np.asarray(position_ids)

    nc = _get_nc()
    in_maps = []
    for c in range(NCORES):
        b, hh = c // 2, c % 2
        rows = slice(b * S, (b + 1) * S)
        qsl = slice(hh * HPC * HD, (hh + 1) * HPC * HD)
        ksl = slice(N_EMBD + hh * HPC * HD, N_EMBD + (hh + 1) * HPC * HD)
        vsl = slice(2 * N_EMBD + hh * HPC * HD, 2 * N_EMBD + (hh + 1) * HPC * HD)
        wqkvT_c = np.concatenate(
            [Wqkv[qsl], Wqkv[ksl], Wqkv[vsl]], axis=0).T
        # own heads' contraction rows (Wo columns), all 1024 out columns
        woT_c = Wo[:, hh * HPC * HD:(hh + 1) * HPC * HD].T
        pos = position_ids[rows]
        in_maps.append({
            'xT': np.ascontiguousarray(x[rows].T).astype(bf16),
            'wqkvT': np.ascontiguousarray(wqkvT_c).astype(bf16),
            'woT': np.ascontiguousarray(woT_c).astype(bf16),
            'cosg': np.ascontiguousarray(cos_cache[pos]).astype(bf16),
            'sing': np.ascontiguousarray(sin_cache[pos]).astype(bf16),
        })

    r = run_bass_kernel_spmd(nc, in_maps, list(range(NCORES)))
    out = np.empty((N, N_EMBD), dtype=np.float32)
    for b in range(B):
        rows = slice(b * S, (b + 1) * S)
        out[rows] = (np.asarray(r.results[2 * b]['ypart']).astype(np.float32) +
                     np.asarray(r.results[2 * b + 1]['ypart']).astype(np.float32))
    _cached['last_results'] = r
    return out


# revision 61
# speedup vs baseline: 1.1506x; 1.0062x over previous
"""Causal varlen self-attention (packed equal-length sequences) on 8 trn2 cores.

Sharding: 4 sequences x 2 head-groups. Core c handles sequence b = c//2 and
heads hh*8..hh*8+8 (hh = c%2). Each core computes the QKV projection of its
sequence restricted to its 8 heads, rotary+RMSNorm, causal attention for all
1024 rows over its heads, exchanges bf16 attention outputs with its pair
partner via AllGather, and computes the final output projection for its
512-wide column slice of y (even core: out cols 0..512, odd: 512..1024) over
the full 1024-feature contraction. The host assembles y column-wise -- the
program is SPMD-symmetric with no all-reduce.

All matmul inputs are bf16 (f32 PSUM accumulation); the host pre-transposes x
and pre-converts weights. Scores are computed in a transposed layout
[kpos, q] with causal column-trimming (matmuls/exp/PV only touch q >= kc*128);
within-diagonal-block masking zeroes est's upper triangle on the gpsimd
engine after exp. The k-side RMS norm is folded into exp's per-partition
scale; softmax denominators come from 64 ones-columns appended to V and one
vector divide per head normalizes the attention output.
"""
import numpy as np

N_EMBD = 1024
N_HEAD = 16
HD = 64
S = 1024
B = 4
N = B * S
NCORES = 8
HPC = 8            # heads per core
NHC = HPC // 2     # head-pair chunks per core
NB = S // 128      # row blocks per sequence
ND = N_EMBD // 128  # contraction chunks
JW = 3 * HPC * HD  # qkv feature width per core (1536)
OW = N_EMBD // 2   # output columns per core (512)
RMS_EPS = 1.1920929e-07

_cached = {}


def _build():
    import concourse.bacc as bacc
    import concourse.mybir as mybir
    import concourse.tile as tile
    import concourse.bass as bass
    from concourse.masks import make_identity

    F32 = mybir.dt.float32
    BF16 = mybir.dt.bfloat16
    ALU = mybir.AluOpType
    ACT = mybir.ActivationFunctionType

    nc = bacc.Bacc('TRN2', target_bir_lowering=False, debug=False,
                   num_devices=NCORES)
    xT = nc.dram_tensor('xT', [N_EMBD, S], BF16, kind='ExternalInput').ap()
    wqkvT = nc.dram_tensor('wqkvT', [N_EMBD, JW], BF16, kind='ExternalInput').ap()
    woT = nc.dram_tensor('woT', [NHC * 128, N_EMBD], BF16, kind='ExternalInput').ap()
    cosg = nc.dram_tensor('cosg', [S, HD // 2], BF16, kind='ExternalInput').ap()
    sing = nc.dram_tensor('sing', [S, HD // 2], BF16, kind='ExternalInput').ap()
    # partial y over this core's 8 heads (all 1024 out cols); host sums pairs
    ypart = nc.dram_tensor('ypart', [S, N_EMBD], BF16, kind='ExternalOutput').ap()

    def bcast_mid(t, n):
        # view [128, w] tile as [128, n, w] broadcasting over middle dim
        return bass.AP(tensor=t.tensor, offset=t.offset,
                       ap=[t.ap[0], [0, n], t.ap[-1]])

    def bcast_last(t, width):
        # view [128, n] tile as [128, n, width] broadcasting over last dim
        return bass.AP(tensor=t.tensor, offset=t.offset,
                       ap=[t.ap[0], t.ap[1], [0, width]])

    with tile.TileContext(nc) as tc:
        import contextlib
        ctx = contextlib.ExitStack()
        with ctx:
            const = ctx.enter_context(tc.tile_pool(name='const', bufs=1))
            persist = ctx.enter_context(tc.tile_pool(name='persist', bufs=1))

            ident = const.tile([128, 128], BF16)
            make_identity(nc, ident)
            epst = const.tile([128, 1], F32)
            nc.vector.memset(epst, RMS_EPS)
            # multiplicative causal mask for diagonal blocks: 1 where q >= k
            trimask = const.tile([128, 128], BF16)
            nc.gpsimd.memset(trimask, 1.0)
            nc.gpsimd.affine_select(
                out=trimask, in_=trimask, compare_op=ALU.is_ge,
                fill=0.0, base=0, pattern=[[1, 128]], channel_multiplier=-1)

            # persistent SBUF data
            xTs = [persist.tile([128, S], BF16, name=f'xTs{d}') for d in range(ND)]
            wq = [persist.tile([128, JW], BF16, name=f'wq{d}') for d in range(ND)]
            wo = [persist.tile([128, N_EMBD], BF16, name=f'wo{f}') for f in range(NHC)]
            cosb = [const.tile([128, HD // 2], BF16, name=f'cos{i}') for i in range(NB)]
            sinb = [const.tile([128, HD // 2], BF16, name=f'sin{i}') for i in range(NB)]
            qT = [persist.tile([128, S], BF16, name=f'qT{i}') for i in range(NHC)]
            kT = [persist.tile([128, S], BF16, name=f'kT{i}') for i in range(NHC)]
            vt = [persist.tile([128, HPC, 128], BF16, name=f'vt{i}') for i in range(NB)]
            attT = [persist.tile([128, S], BF16, name=f'attT{f}') for f in range(NHC)]
            # rnkt[nb][:, 0:8] = q-norm recip (with HD^-0.5), [:, 8:16] = k-norm recip
            rnkt = [persist.tile([128, N_HEAD], F32, name=f'rn{i}') for i in range(NB)]

            # prologue DMAs, interleaved so block-0 work can start early
            for d in range(ND):
                nc.sync.dma_start(out=xTs[d], in_=xT[d * 128:(d + 1) * 128])
                nc.sync.dma_start(out=wq[d], in_=wqkvT[d * 128:(d + 1) * 128])
            for nb in range(NB):
                nc.sync.dma_start(out=cosb[nb], in_=cosg[nb * 128:(nb + 1) * 128])
                nc.sync.dma_start(out=sinb[nb], in_=sing[nb * 128:(nb + 1) * 128])
            for f in range(NHC):
                nc.sync.dma_start(out=wo[f], in_=woT[f * 128:(f + 1) * 128])
            for nb in range(NB):
                nc.gpsimd.memset(vt[nb][:, :, 0:HD], 1.0)

            # ---- phase 1: QKV projection + rotary + rms + transposes ----
            # The q/k transposes are deferred until after all 8 blocks' QKV
            # matmuls so the PE sees one long uninterrupted stream (p-state).
            rott = [persist.tile([128, N_HEAD, HD], BF16, name=f'rot{i}')
                    for i in range(NB)]
            with tc.tile_pool(name='qkw', bufs=3) as qkw, \
                 tc.tile_pool(name='rotw', bufs=3) as rotw, \
                 tc.tile_pool(name='psq', bufs=2, space='PSUM') as psq, \
                 tc.tile_pool(name='ptr', bufs=2, space='PSUM') as ptr:
                for nb in range(NB):
                    rsl = slice(nb * 128, (nb + 1) * 128)
                    pq = psq.tile([128, 3 * HPC, HD], F32, tag='pq')
                    for d in range(ND):
                        for g in range(3):
                            nc.tensor.matmul(
                                pq[:, g * HPC:(g + 1) * HPC],
                                xTs[d][:, rsl],
                                wq[d][:, g * 512:(g + 1) * 512],
                                start=(d == 0), stop=(d == ND - 1))
                    # evacuate psum: q,k -> bf16 for rotary; v -> vt
                    qk = qkw.tile([128, N_HEAD, HD], BF16, tag='qk')
                    nc.scalar.copy(qk, pq[:, 0:N_HEAD])
                    nc.scalar.copy(vt[nb][:, :, HD:128], pq[:, N_HEAD:3 * HPC])

                    # rotary on q+k heads together (all bf16, 2x DVE)
                    cb = bcast_mid(cosb[nb], N_HEAD)
                    sb = bcast_mid(sinb[nb], N_HEAD)
                    x1 = qk[:, :, 0:32]
                    x2 = qk[:, :, 32:64]
                    rot = rott[nb]
                    scr = rotw.tile([128, N_HEAD, HD], BF16, tag='scr')
                    nc.vector.tensor_tensor(out=rot[:, :, 0:32], in0=x1, in1=cb, op=ALU.mult)
                    nc.vector.tensor_tensor(out=scr[:, :, 0:32], in0=x2, in1=sb, op=ALU.mult)
                    nc.vector.tensor_tensor(out=rot[:, :, 0:32], in0=rot[:, :, 0:32],
                                            in1=scr[:, :, 0:32], op=ALU.add)
                    nc.vector.tensor_tensor(out=rot[:, :, 32:64], in0=x2, in1=cb, op=ALU.mult)
                    nc.vector.tensor_tensor(out=scr[:, :, 32:64], in0=x1, in1=sb, op=ALU.mult)
                    nc.vector.tensor_tensor(out=rot[:, :, 32:64], in0=rot[:, :, 32:64],
                                            in1=scr[:, :, 32:64], op=ALU.subtract)

                    # rms: ms = sum(rot^2) over head dim; rn = 1/sqrt(ms/64+eps)
                    nc.vector.tensor_tensor(out=scr, in0=rot, in1=rot, op=ALU.mult)
                    ms = qkw.tile([128, N_HEAD], F32, tag='ms')
                    nc.vector.reduce_sum(out=ms, in_=scr, axis=mybir.AxisListType.X)
                    nc.scalar.activation(out=ms, in_=ms, func=ACT.Sqrt,
                                         bias=epst, scale=1.0 / HD)
                    nc.vector.reciprocal(out=rnkt[nb], in_=ms)
                    # fold HD^-0.5 into the q-side recips (bf16 for the mul)
                    rnq = qkw.tile([128, HPC], BF16, tag='rnq')
                    nc.scalar.mul(out=rnq, in_=rnkt[nb][:, 0:HPC], mul=HD ** -0.5)
                    nc.vector.tensor_tensor(out=rot[:, 0:HPC, :], in0=rot[:, 0:HPC, :],
                                            in1=bcast_last(rnq, HD), op=ALU.mult)

                # transpose q (normalized) and k (unnormalized) head-pairs
                for nb in range(NB):
                    rsl = slice(nb * 128, (nb + 1) * 128)
                    rot = rott[nb]
                    for hc in range(NHC):
                        pt = ptr.tile([128, 128], BF16, tag='pt')
                        nc.tensor.transpose(
                            pt, rot[:, 2 * hc:2 * hc + 2, :].rearrange("p a b -> p (a b)"),
                            ident)
                        nc.vector.tensor_copy(qT[hc][:, rsl], pt)
                        pt2 = ptr.tile([128, 128], BF16, tag='pt')
                        nc.tensor.transpose(
                            pt2, rot[:, HPC + 2 * hc:HPC + 2 * hc + 2, :].rearrange("p a b -> p (a b)"),
                            ident)
                        nc.scalar.copy(kT[hc][:, rsl], pt2)

            # ---- phase 2: attention (scores^T -> exp -> mask -> PV) ----
            with tc.tile_pool(name='estp', bufs=4) as estp, \
                 tc.tile_pool(name='pssc', bufs=2, space='PSUM') as pssc, \
                 tc.tile_pool(name='pspv', bufs=2, space='PSUM') as pspv:
                for h in range(HPC):
                    hc, h2 = h // 2, h % 2
                    dsl = slice(h2 * HD, (h2 + 1) * HD)
                    pv = pspv.tile([128, S], F32, tag='pv')

                    def chunks_of(kc):
                        c0 = kc * 128
                        return [(c0, 512), (512, S)] if c0 < 512 else [(c0, S)]

                    def scores(kc):
                        # returns est tile; emits scores matmuls + exp + mask
                        c0 = kc * 128
                        sct = pssc.tile([128, S], F32, tag='sct')
                        for a, b in chunks_of(kc):
                            nc.tensor.matmul(
                                sct[:, a:b],
                                kT[hc][dsl, c0:c0 + 128],
                                qT[hc][dsl, a:b],
                                start=True, stop=True)
                        est = estp.tile([128, S], BF16, tag='est')
                        nc.scalar.activation(out=est[:, c0:], in_=sct[:, c0:],
                                             func=ACT.Exp,
                                             scale=rnkt[kc][:, HPC + h:HPC + h + 1])
                        # zero the strictly-upper triangle of the diagonal block
                        nc.vector.tensor_tensor(
                            out=est[:, c0:c0 + 128], in0=est[:, c0:c0 + 128],
                            in1=trimask, op=ALU.mult)
                        return est

                    # software-pipelined emission: the PE queue is in-order, so
                    # keep scores(kc+1) AHEAD of pv(kc) -- pv waits on exp+mask
                    ests = {0: scores(0)}
                    for kc in range(NB):
                        if kc + 1 < NB:
                            ests[kc + 1] = scores(kc + 1)
                        est = ests.pop(kc)
                        for a, b in chunks_of(kc):
                            nc.tensor.matmul(
                                pv[:, a:b], vt[kc][:, h], est[:, a:b],
                                start=(kc == 0), stop=(kc == NB - 1),
                                skip_group_check=True)
                    # normalize by the ones-row denominators (pv rows 0:64 --
                    # ones-pad comes FIRST so the approx reciprocal sees a
                    # partition-0 input, which it requires)
                    rden = estp.tile([HD, S], F32, tag='rden')
                    nc.vector.reciprocal_approx_fast(out=rden, in_=pv[0:HD, :])
                    nc.vector.tensor_tensor(out=attT[hc][dsl, :], in0=pv[HD:128, :],
                                            in1=rden, op=ALU.mult)

            # ---- phase 3: output projection over all 16 heads ----
            with tc.tile_pool(name='yw', bufs=4) as yw, \
                 tc.tile_pool(name='psy', bufs=8, space='PSUM') as psy:
                # partial y over own 4 head-pairs, all 1024 out cols, emitted
                # slot-outer (head-pair outer) so row-blocks' matmuls for the
                # early head-pairs run while the last heads' attention finishes
                for og in range(2):
                    osl = slice(og * 512, (og + 1) * 512)
                    py = [psy.tile([128, 512], F32, tag='py', name=f'py{og}_{qt}')
                          for qt in range(NB)]
                    for f in range(NHC):
                        for qt in range(NB):
                            nc.tensor.matmul(
                                py[qt], attT[f][:, qt * 128:(qt + 1) * 128],
                                wo[f][:, osl],
                                start=(f == 0), stop=(f == NHC - 1))
                    for qt in range(NB):
                        ys = yw.tile([128, 512], BF16, tag='ys')
                        if qt % 2 == 0:
                            nc.vector.tensor_copy(ys, py[qt])
                        else:
                            nc.scalar.copy(ys, py[qt])
                        nc.sync.dma_start(
                            out=ypart[qt * 128:(qt + 1) * 128, osl], in_=ys)

    nc.compile()
    return nc


def _get_nc():
    if 'nc' not in _cached:
        _cached['nc'] = _build()
    return _cached['nc']


def kernel(x, Wqkv, Wo, cos_cache, sin_cache, cu_seqlens, position_ids,
           max_seqlen, **_ignored):
    from concourse.bass_utils import run_bass_kernel_spmd
    import ml_dtypes

    bf16 = ml_dtypes.bfloat16
    x = np.asarray(x, dtype=np.float32)
    Wqkv = np.asarray(Wqkv, dtype=np.float32)
    Wo = np.asarray(Wo, dtype=np.float32)
    cos_cache = np.asarray(cos_cache, dtype=np.float32)
    sin_cache = np.asarray(sin_cache, dtype=np.float32)
    position_ids = np.asarray(position_ids)

    nc = _get_nc()
    in_maps = []
    for c in range(NCORES):
        b, hh = c // 2, c % 2
        rows = slice(b * S, (b + 1) * S)
        qsl = slice(hh * HPC * HD, (hh + 1) * HPC * HD)
        ksl = slice(N_EMBD + hh * HPC * HD, N_EMBD + (hh + 1) * HPC * HD)
        vsl = slice(2 * N_EMBD + hh * HPC * HD, 2 * N_EMBD + (hh + 1) * HPC * HD)
        wqkvT_c = np.concatenate(
            [Wqkv[qsl], Wqkv[ksl], Wqkv[vsl]], axis=0).T
        # own heads' contraction rows (Wo columns), all 1024 out columns
        woT_c = Wo[:, hh * HPC * HD:(hh + 1) * HPC * HD].T
        pos = position_ids[rows]
        in_maps.append({
            'xT': np.ascontiguousarray(x[rows].T).astype(bf16),
            'wqkvT': np.ascontiguousarray(wqkvT_c).astype(bf16),
            'woT': np.ascontiguousarray(woT_c).astype(bf16),
            'cosg': np.ascontiguousarray(cos_cache[pos]).astype(bf16),
            'sing': np.ascontiguousarray(sin_cache[pos]).astype(bf16),
        })

    r = run_bass_kernel_spmd(nc, in_maps, list(range(NCORES)))
    out = np.empty((N, N_EMBD), dtype=np.float32)
    for b in range(B):
        rows = slice(b * S, (b + 1) * S)
        out[rows] = (np.asarray(r.results[2 * b]['ypart']).astype(np.float32) +
                     np.asarray(r.results[2 * b + 1]['ypart']).astype(np.float32))
    _cached['last_results'] = r
    return out
